# revision 8
# baseline (speedup 1.0000x reference)
"""Luong attention (method='general') scores for batch — TRN2 Bass kernel.

Reference computation (jax):
    proj   = einsum('sbh,oh->sbo', encoder_outputs, attn_w) + attn_b   # [S,B,H]
    scores = einsum('bh,sbh->bs', hidden[0], proj)                      # [B,S]
    attn   = softmax(scores, axis=1)                                    # [B,S]

Algebraic rewrite: scores[b,s] = enc[s,b,:]·q[b,:] with q = hidden[0]@attn_w
(host-side, 67 MFLOP). The bias term is constant in s and cancels in softmax.

Device kernel (per core, data-parallel over batch, 4 batches/core):
  - enc shard shipped as fp16 (16 MB/core — kernel is DMA-bound, so half of
    f32) in h-major chunks [hc][128 hp][b][s]: each of the 8 h-chunks is a
    fully contiguous 2 MB DMA.
  - the dot products run on the PE: per (hc, b, s-block) one self-loading
    matmul with the enc block [128h, 128s] as stationary and q[hc,b] [128,1]
    moving, accumulating over the 8 h-chunks into PSUM scores [128, (b,sb)]
    (f32, exact). ~30us of PE time under ~50us of DMA; DVE/Act stay free.
  - last h-chunk is DMA'd per-batch so its matmuls pipeline with the tail.
  - softmax tail without gpsimd all-reduces (14.5us in the v1 tail):
    per-partition maxes ride through a single PE transpose next to the
    scores; per-batch max / exp-sum are broadcast back across the 64 (b,sb)
    partitions with two tiny mask matmuls on the PE. Output is written from
    the transposed layout directly.
"""

import numpy as np

import concourse.bacc as bacc
import concourse.bass as bass
import concourse.bass_isa as bass_isa
import concourse.mybir as mybir
import concourse.tile as tile
from concourse.bass_utils import run_bass_kernel_spmd
from concourse.masks import make_identity

F32 = mybir.dt.float32
F16 = mybir.dt.float16

S, B, H = 2048, 32, 1024
NCORES = 8
BL = B // NCORES        # batches per core = 4
T = S // 128            # s-blocks of 128 = 16
HC = H // 128           # h-chunks = 8

_CACHE: dict = {}


def _build_program():
    nc = bacc.Bacc(
        "TRN2",
        target_bir_lowering=False,
        debug=False,
        enable_asserts=True,
        num_devices=NCORES,
    )
    enc = nc.dram_tensor("enc", [HC, 128, BL * S], F16, kind="ExternalInput").ap()
    q = nc.dram_tensor("q", [128, HC * BL], F16, kind="ExternalInput").ap()
    # masks[:, 0:64]  = sumrep  [(b,t),(b',t')] = 1 if b==b'
    # masks[0:4, 64:128] = negmask4 [b,(b',t')] = -1 if b==b'
    masks = nc.dram_tensor("masks", [64, 128], F32, kind="ExternalInput").ap()
    out = nc.dram_tensor("out", [BL, S], F32, kind="ExternalOutput").ap()

    maxop = mybir.AluOpType.max

    with tile.TileContext(nc) as tc:
        with (
            tc.tile_pool(name="consts", bufs=1) as consts,
            tc.tile_pool(name="encp", bufs=3) as encp,
            tc.tile_pool(name="small", bufs=1) as small,
            tc.tile_pool(name="pst", bufs=1, space="PSUM") as pst,
        ):
            # ---- constants / q, off the sync ring so enc streams first ----
            qt = consts.tile([128, HC, BL], F16)
            nc.scalar.dma_start(out=qt, in_=q.rearrange("p (c b) -> p c b", b=BL))
            masks_sb = consts.tile([64, 128], F32)
            nc.scalar.dma_start(out=masks_sb, in_=masks)
            identity = consts.tile([128, 128], F32)
            make_identity(nc, identity)

            psum_sc = pst.tile([128, BL * T], F32, tag="scores")

            # PSUM start/stop semantics: start=True marks the whole 2KB bank
            # pending-zero, and each column's first-touch write zeroes itself.
            # So only the globally-first matmul starts the group and only the
            # globally-last one stops it; everything between accumulates.
            NMM = HC * BL * T
            mm_idx = [0]

            def hc_matmuls(hc, et, batches):
                for b in batches:
                    for sb in range(T):
                        col = b * T + sb
                        m = mm_idx[0]
                        mm_idx[0] += 1
                        nc.tensor.matmul(
                            psum_sc[:, col : col + 1],
                            lhsT=et[:, b, sb * 128 : (sb + 1) * 128],
                            rhs=qt[:, hc, b : b + 1],
                            start=(m == 0),
                            stop=(m == NMM - 1),
                        )

            # ---- streaming pass over enc (h-major chunks) -----------------
            for hc in range(HC - 1):
                et = encp.tile([128, BL, S], F16)
                nc.sync.dma_start(
                    out=et, in_=enc[hc].rearrange("p (b s) -> p b s", b=BL)
                )
                hc_matmuls(hc, et, range(BL))

            # last h-chunk: per-batch DMAs so matmuls pipeline with the tail;
            # the final batch lands in s-quarters so the last matmuls trail
            # the last 128KB of DMA instead of the last 512KB.
            hc = HC - 1
            et_l = encp.tile([128, BL, S], F16, tag="enclast", bufs=1)
            enc_l = enc[hc].rearrange("p (b s) -> p b s", b=BL)
            for b in range(BL - 1):
                nc.sync.dma_start(out=et_l[:, b], in_=enc_l[:, b])
                hc_matmuls(hc, et_l, [b])
            b = BL - 1
            for qq in range(4):
                sl = slice(qq * (S // 4), (qq + 1) * (S // 4))
                nc.sync.dma_start(out=et_l[:, b, sl], in_=enc_l[:, b, sl])
                for sb in range(qq * (T // 4), (qq + 1) * (T // 4)):
                    col = b * T + sb
                    m = mm_idx[0]
                    mm_idx[0] += 1
                    nc.tensor.matmul(
                        psum_sc[:, col : col + 1],
                        lhsT=et_l[:, b, sb * 128 : (sb + 1) * 128],
                        rhs=qt[:, hc, b : b + 1],
                        start=(m == 0),
                        stop=(m == NMM - 1),
                    )

            # ---- softmax over s (per batch), transposed-domain tail -------
            # scomb: scores [128, (b t)] in cols 0:64, per-partition maxes in
            # cols 64:68 — transposed together in one PE op.
            scomb = small.tile([128, 68], F32)
            nc.vector.tensor_reduce(
                out=scomb[:, 64:68],
                in_=psum_sc.rearrange("p (j t) -> p j t", t=T),
                axis=mybir.AxisListType.X,
                op=maxop,
            )
            nc.scalar.copy(out=scomb[:, 0:64], in_=psum_sc)
            st_ps = pst.tile([68, 128], F32, tag="st")
            nc.tensor.transpose(st_ps, scomb, identity)
            # per-batch global max (4 values) from the transposed pmax rows
            bmax4 = small.tile([4, 1], F32, tag="bmax4")
            nc.vector.tensor_reduce(
                out=bmax4, in_=st_ps[64:68, :], axis=mybir.AxisListType.X, op=maxop
            )
            # broadcast -max(b) to all 16 (b,t) partitions via mask matmul
            negb_ps = pst.tile([64, 1], F32, tag="negb")
            nc.tensor.matmul(negb_ps, lhsT=masks_sb[0:4, 64:128], rhs=bmax4)
            negb64 = small.tile([64, 1], F32, tag="negb64")
            nc.vector.tensor_copy(out=negb64, in_=negb_ps)
            # exp(score - bmax) and per-(b,t) partial sums in one Act op
            probs_t = small.tile([64, 128], F32, tag="probs")
            esum64 = small.tile([64, 1], F32, tag="esum")
            nc.scalar.activation(
                out=probs_t,
                in_=st_ps[0:64, :],
                func=mybir.ActivationFunctionType.Exp,
                bias=negb64,
                accum_out=esum64,
            )
            # per-batch total sum, replicated to all (b,t) partitions
            dsum_ps = pst.tile([64, 1], F32, tag="dsum")
            nc.tensor.matmul(dsum_ps, lhsT=masks_sb[:, 0:64], rhs=esum64)
            rsum64 = small.tile([64, 1], F32, tag="rsum")
            nc.vector.reciprocal(out=rsum64, in_=dsum_ps)
            attn_sb = small.tile([64, 128], F32, tag="attn")
            nc.scalar.mul(attn_sb, probs_t, rsum64)
            nc.sync.dma_start(
                out=out.rearrange("b (t s) -> (b t) s", s=128), in_=attn_sb
            )

    nc.compile()
    return nc


def _make_masks():
    m = np.zeros((64, 128), dtype=np.float32)
    bt = np.arange(64) // T  # batch of each (b,t) partition
    m[:, 0:64] = (bt[:, None] == bt[None, :]).astype(np.float32)
    m[0:4, 64:128] = -(np.arange(4)[:, None] == bt[None, :]).astype(np.float32)
    return m


def _shard_inputs(hidden, encoder_outputs, attn_w):
    # torch-Linear convention: proj = enc @ W^T, so q = hidden @ W.
    qfull = (hidden[0].astype(np.float32) @ attn_w.astype(np.float32)).astype(
        np.float16
    )
    masks = _make_masks()
    in_maps = []
    for i in range(NCORES):
        bs = slice(i * BL, (i + 1) * BL)
        # [S, BL, H] -> [hc, hp, b, s] fp16, contiguous per h-chunk
        e = np.ascontiguousarray(encoder_outputs[:, bs, :]).astype(np.float16)
        e = e.reshape(S, BL, HC, 128).transpose(2, 3, 1, 0)
        enc_i = np.ascontiguousarray(e).reshape(HC, 128, BL * S)
        # q [BL, H] -> [hp, hc, b]
        qt_i = np.ascontiguousarray(
            qfull[bs].reshape(BL, HC, 128).transpose(2, 1, 0)
        ).reshape(128, HC * BL)
        in_maps.append({"enc": enc_i, "q": qt_i, "masks": masks})
    return in_maps


def kernel(hidden, encoder_outputs, attn_w, attn_b):
    if "nc" not in _CACHE:
        _CACHE["nc"] = _build_program()
    nc = _CACHE["nc"]

    hidden = np.asarray(hidden, dtype=np.float32)
    encoder_outputs = np.asarray(encoder_outputs, dtype=np.float32)
    attn_w = np.asarray(attn_w, dtype=np.float32)

    in_maps = _shard_inputs(hidden, encoder_outputs, attn_w)
    # Run twice and return the second result: a crashed prior process can
    # leave device semaphores nonzero, corrupting the first execution; the
    # kernel's own epilogue clears them, so the second run starts clean.
    run_bass_kernel_spmd(nc, in_maps, core_ids=list(range(NCORES)))
    res = run_bass_kernel_spmd(nc, in_maps, core_ids=list(range(NCORES)))
    attn = np.concatenate([res.results[i]["out"] for i in range(NCORES)], axis=0)
    return attn[None].astype(np.float32)


# revision 10
# speedup vs baseline: 1.0126x; 1.0126x over previous
"""Luong attention (method='general') scores for batch — TRN2 Bass kernel.

Reference computation (jax):
    proj   = einsum('sbh,oh->sbo', encoder_outputs, attn_w) + attn_b   # [S,B,H]
    scores = einsum('bh,sbh->bs', hidden[0], proj)                      # [B,S]
    attn   = softmax(scores, axis=1)                                    # [B,S]

Algebraic rewrite: scores[b,s] = enc[s,b,:]·q[b,:] with q = hidden[0]@attn_w
(host-side, 67 MFLOP). The bias term is constant in s and cancels in softmax.

Device kernel (per core, data-parallel over batch, 4 batches/core):
  - enc shard shipped as fp16 (16 MB/core — kernel is DMA-bound, so half of
    f32) in h-major chunks [hc][128 hp][b][s]: each of the 8 h-chunks is a
    fully contiguous 2 MB DMA.
  - the dot products run on the PE: per (hc, b, s-block) one self-loading
    matmul with the enc block [128h, 128s] as stationary and q[hc,b] [128,1]
    moving, accumulating over the 8 h-chunks into PSUM scores [128, (b,sb)]
    (f32, exact). ~30us of PE time under ~50us of DMA; DVE/Act stay free.
  - last h-chunk is DMA'd per-batch so its matmuls pipeline with the tail.
  - softmax tail without gpsimd all-reduces (14.5us in the v1 tail):
    per-partition maxes ride through a single PE transpose next to the
    scores; per-batch max / exp-sum are broadcast back across the 64 (b,sb)
    partitions with two tiny mask matmuls on the PE. Output is written from
    the transposed layout directly.
"""

import numpy as np

import concourse.bacc as bacc
import concourse.bass as bass
import concourse.bass_isa as bass_isa
import concourse.mybir as mybir
import concourse.tile as tile
from concourse.bass_utils import run_bass_kernel_spmd
from concourse.masks import make_identity

F32 = mybir.dt.float32
F16 = mybir.dt.float16

S, B, H = 2048, 32, 1024
NCORES = 8
BL = B // NCORES        # batches per core = 4
T = S // 128            # s-blocks of 128 = 16
HC = H // 128           # h-chunks = 8

_CACHE: dict = {}


def _build_program():
    nc = bacc.Bacc(
        "TRN2",
        target_bir_lowering=False,
        debug=False,
        enable_asserts=True,
        num_devices=NCORES,
    )
    enc = nc.dram_tensor("enc", [HC, 128, BL * S], F16, kind="ExternalInput").ap()
    q = nc.dram_tensor("q", [128, HC * BL], F16, kind="ExternalInput").ap()
    # masks[:, 0:64]  = sumrep  [(b,t),(b',t')] = 1 if b==b'
    # masks[0:4, 64:128] = negmask4 [b,(b',t')] = -1 if b==b'
    masks = nc.dram_tensor("masks", [64, 128], F32, kind="ExternalInput").ap()
    out = nc.dram_tensor("out", [BL, S], F32, kind="ExternalOutput").ap()

    maxop = mybir.AluOpType.max

    with tile.TileContext(nc) as tc:
        with (
            tc.tile_pool(name="consts", bufs=1) as consts,
            tc.tile_pool(name="encp", bufs=3) as encp,
            tc.tile_pool(name="small", bufs=1) as small,
            tc.tile_pool(name="pst", bufs=1, space="PSUM") as pst,
        ):
            # ---- constants / q, off the sync ring so enc streams first ----
            qt = consts.tile([128, HC, BL], F16)
            nc.scalar.dma_start(out=qt, in_=q.rearrange("p (c b) -> p c b", b=BL))
            masks_sb = consts.tile([64, 128], F32)
            nc.scalar.dma_start(out=masks_sb, in_=masks)
            identity = consts.tile([128, 128], F32)
            make_identity(nc, identity)

            psum_sc = pst.tile([128, BL * T], F32, tag="scores")

            # PSUM start/stop semantics: start=True marks the whole 2KB bank
            # pending-zero, and each column's first-touch write zeroes itself.
            # So only the globally-first matmul starts the group and only the
            # globally-last one stops it; everything between accumulates.
            NMM = HC * BL * T
            mm_idx = [0]

            def hc_matmuls(hc, et, batches):
                for b in batches:
                    for sb in range(T):
                        col = b * T + sb
                        m = mm_idx[0]
                        mm_idx[0] += 1
                        nc.tensor.matmul(
                            psum_sc[:, col : col + 1],
                            lhsT=et[:, b, sb * 128 : (sb + 1) * 128],
                            rhs=qt[:, hc, b : b + 1],
                            start=(m == 0),
                            stop=(m == NMM - 1),
                        )

            # ---- streaming pass over enc (h-major chunks) -----------------
            for hc in range(HC - 1):
                et = encp.tile([128, BL, S], F16)
                nc.sync.dma_start(
                    out=et, in_=enc[hc].rearrange("p (b s) -> p b s", b=BL)
                )
                hc_matmuls(hc, et, range(BL))

            # last h-chunk: per-batch DMAs so matmuls pipeline with the tail
            hc = HC - 1
            et_l = encp.tile([128, BL, S], F16, tag="enclast", bufs=1)
            enc_l = enc[hc].rearrange("p (b s) -> p b s", b=BL)
            for b in range(BL):
                nc.sync.dma_start(out=et_l[:, b], in_=enc_l[:, b])
                hc_matmuls(hc, et_l, [b])

            # ---- softmax over s (per batch), transposed-domain tail -------
            # scomb: scores [128, (b t)] in cols 0:64, per-partition maxes in
            # cols 64:68 — transposed together in one PE op.
            scomb = small.tile([128, 68], F32)
            nc.vector.tensor_reduce(
                out=scomb[:, 64:68],
                in_=psum_sc.rearrange("p (j t) -> p j t", t=T),
                axis=mybir.AxisListType.X,
                op=maxop,
            )
            nc.scalar.copy(out=scomb[:, 0:64], in_=psum_sc)
            st_ps = pst.tile([68, 128], F32, tag="st")
            nc.tensor.transpose(st_ps, scomb, identity)
            # per-batch global max (4 values) from the transposed pmax rows
            bmax4 = small.tile([4, 1], F32, tag="bmax4")
            nc.vector.tensor_reduce(
                out=bmax4, in_=st_ps[64:68, :], axis=mybir.AxisListType.X, op=maxop
            )
            # broadcast -max(b) to all 16 (b,t) partitions via mask matmul
            negb_ps = pst.tile([64, 1], F32, tag="negb")
            nc.tensor.matmul(negb_ps, lhsT=masks_sb[0:4, 64:128], rhs=bmax4)
            negb64 = small.tile([64, 1], F32, tag="negb64")
            nc.vector.tensor_copy(out=negb64, in_=negb_ps)
            # exp(score - bmax) and per-(b,t) partial sums in one Act op
            probs_t = small.tile([64, 128], F32, tag="probs")
            esum64 = small.tile([64, 1], F32, tag="esum")
            nc.scalar.activation(
                out=probs_t,
                in_=st_ps[0:64, :],
                func=mybir.ActivationFunctionType.Exp,
                bias=negb64,
                accum_out=esum64,
            )
            # per-batch total sum, replicated to all (b,t) partitions
            dsum_ps = pst.tile([64, 1], F32, tag="dsum")
            nc.tensor.matmul(dsum_ps, lhsT=masks_sb[:, 0:64], rhs=esum64)
            rsum64 = small.tile([64, 1], F32, tag="rsum")
            nc.vector.reciprocal(out=rsum64, in_=dsum_ps)
            attn_sb = small.tile([64, 128], F32, tag="attn")
            # DVE, not Act: runs right after its own reciprocal with no
            # cross-engine hop, and skips Act's long SBUF-access init.
            nc.vector.tensor_scalar_mul(out=attn_sb, in0=probs_t, scalar1=rsum64)
            nc.sync.dma_start(
                out=out.rearrange("b (t s) -> (b t) s", s=128), in_=attn_sb
            )

    nc.compile()
    return nc


def _make_masks():
    m = np.zeros((64, 128), dtype=np.float32)
    bt = np.arange(64) // T  # batch of each (b,t) partition
    m[:, 0:64] = (bt[:, None] == bt[None, :]).astype(np.float32)
    m[0:4, 64:128] = -(np.arange(4)[:, None] == bt[None, :]).astype(np.float32)
    return m


def _shard_inputs(hidden, encoder_outputs, attn_w):
    # torch-Linear convention: proj = enc @ W^T, so q = hidden @ W.
    qfull = (hidden[0].astype(np.float32) @ attn_w.astype(np.float32)).astype(
        np.float16
    )
    masks = _make_masks()
    in_maps = []
    for i in range(NCORES):
        bs = slice(i * BL, (i + 1) * BL)
        # [S, BL, H] -> [hc, hp, b, s] fp16, contiguous per h-chunk
        e = np.ascontiguousarray(encoder_outputs[:, bs, :]).astype(np.float16)
        e = e.reshape(S, BL, HC, 128).transpose(2, 3, 1, 0)
        enc_i = np.ascontiguousarray(e).reshape(HC, 128, BL * S)
        # q [BL, H] -> [hp, hc, b]
        qt_i = np.ascontiguousarray(
            qfull[bs].reshape(BL, HC, 128).transpose(2, 1, 0)
        ).reshape(128, HC * BL)
        in_maps.append({"enc": enc_i, "q": qt_i, "masks": masks})
    return in_maps


def kernel(hidden, encoder_outputs, attn_w, attn_b):
    if "nc" not in _CACHE:
        _CACHE["nc"] = _build_program()
    nc = _CACHE["nc"]

    hidden = np.asarray(hidden, dtype=np.float32)
    encoder_outputs = np.asarray(encoder_outputs, dtype=np.float32)
    attn_w = np.asarray(attn_w, dtype=np.float32)

    in_maps = _shard_inputs(hidden, encoder_outputs, attn_w)
    # Run twice and return the second result: a crashed prior process can
    # leave device semaphores nonzero, corrupting the first execution; the
    # kernel's own epilogue clears them, so the second run starts clean.
    run_bass_kernel_spmd(nc, in_maps, core_ids=list(range(NCORES)))
    res = run_bass_kernel_spmd(nc, in_maps, core_ids=list(range(NCORES)))
    attn = np.concatenate([res.results[i]["out"] for i in range(NCORES)], axis=0)
    return attn[None].astype(np.float32)
